# revision 33
# baseline (speedup 1.0000x reference)
"""Trainium2 Bass kernel for nn_LogicConv2d (soft logic-gate conv, difflogic tree).

Strategy
--------
* The reference gather x[:, ac, ah, aw] has sliding-window structure: for each
  (kernel k, leaf slot s) the gathered plane over positions is a shifted
  60x60 crop of x, identified by (channel c, row-shift rh, col-shift rw) with
  rh, rw in [0,5), c in [0,3).  We derive (c, rh, rw) per (k, s, side) on the
  host from the index tensors and verify the structure exactly.
* Shard the K=32 kernels over 8 cores (4 each).  All cores run an identical
  Bass program; per-core differences (crop choices, gate coefficients) are
  pure data: an int32 row-index table driving indirect-DMA gathers, and an
  f32 per-partition scalar table.
* Tree compute: each binop y = c0 + c1*a + c2*b + c3*a*b is folded into
  (a + da) * (c3*b + b1) which materializes V = Y + beta (affine-encoded with
  alpha == 1), beta propagated into the children's constants host-side (f64).
  Per node: 1 ScalarE activation (affine) + 1 VectorE scalar_tensor_tensor.
  Ill-conditioned nodes (|beta| large or c3 ~ 0) get a 3-op safe form; the
  extra op is emitted for every core (g-column = data) to keep SPMD identical.
* On-chip layout: partitions = (b_hi2, out_row60) = 120 lanes, free =
  (b_lo8, out_col60) = 480 f32.  Leaf planes are row-gathers from a DRAM
  table xr3[1920, 480] holding the 5 column-shifted copies of x.
"""

import numpy as np

# ---------------------------------------------------------------- constants
_M = np.array([
    [0, 0, 0, 0], [0, 0, 0, 1], [0, 1, 0, -1], [0, 1, 0, 0],
    [0, 0, 1, -1], [0, 0, 1, 0], [0, 1, 1, -2], [0, 1, 1, -1],
    [1, -1, -1, 1], [1, -1, -1, 2], [1, 0, -1, 0], [1, 0, -1, 1],
    [1, -1, 0, 0], [1, -1, 0, 1], [1, 0, 0, -1], [1, 0, 0, 0],
], dtype=np.float64)

B, C, H, W = 16, 3, 64, 64
K, DEPTH, RF = 32, 4, 5
S = 2 ** DEPTH              # 16
OUT = 60                    # output height == width
P_POS = OUT * OUT           # 3600
NCORES = 8
KPC = K // NCORES           # kernels per core = 4
NLEVELS = DEPTH + 1         # 5
NODES_PER_TREE = 2 ** (DEPTH + 1) - 1   # 31

# device layout
PART = 120                  # partitions used: (b_hi 2) x (out_row 60)
FREE = 480                  # free dim: (b_lo 8) x (out_col 60)
XR_ROWS = RF * C * 2 * H    # 5*3*2*64 = 1920

# numerics
FOLD_TH = 1000.0            # |beta| bound before switching a node to safe form
C3_MIN = 1e-6

# ---------------------------------------------------------------- host math


def _derive_rel(ah, aw, ac):
    """Recover per-(k,s) relative offsets (rh, rw, rc) from absolute index
    tensors; verify the sliding-window structure exactly."""
    oh = (np.arange(P_POS) // OUT).astype(np.int64)   # row of each position
    ow = (np.arange(P_POS) % OUT).astype(np.int64)
    rh = ah[:, 0, :].astype(np.int64)                 # (K, S)
    rw = aw[:, 0, :].astype(np.int64)
    rc = ac[:, 0, :].astype(np.int64)
    ok = (
        np.array_equal(ah, (oh[None, :, None] + rh[:, None, :]).astype(ah.dtype))
        and np.array_equal(aw, (ow[None, :, None] + rw[:, None, :]).astype(aw.dtype))
        and np.array_equal(ac, np.broadcast_to(rc[:, None, :], ac.shape).astype(ac.dtype))
        and rh.min() >= 0 and rh.max() < RF
        and rw.min() >= 0 and rw.max() < RF
        and rc.min() >= 0 and rc.max() < C
    )
    if not ok:
        raise ValueError("index tensors do not have sliding-window structure")
    return rh, rw, rc


def _coef_levels(ws):
    """softmax(w) @ M per level -> list of (n_l, K, 4) float64."""
    out = []
    for w in ws:
        w64 = w.astype(np.float64)
        e = np.exp(w64 - w64.max(axis=-1, keepdims=True))
        p = e / e.sum(axis=-1, keepdims=True)
        out.append(p @ _M)                      # (n_l, K, 4)
    return out


def _fold_tree(coefs):
    """Decide per-node scheme (normal 2-op vs safe 3-op) and compute the
    per-node constants for every kernel k.

    Returns:
      flags: list per level of (n_l,) bool  -- safe-form, unioned over ALL k
             (so the instruction structure is identical for every core).
      params: list per level of dict of (n_l, K) float64 arrays:
             s1 (=c3), b1, da, g  (g only used by safe nodes)
      root_beta: (K,) float64
    """
    flags = []
    params = []
    # beta of the current level's node values, per (node, k); leaves beta=0
    beta = np.zeros((2 * S, K), dtype=np.float64)   # interleaved a/b leaves
    # children of level-l node s are prev[2s] (a) and prev[2s+1] (b).
    # At the leaf level, "prev" is the 32 gathered planes in (a0,b0,a1,b1,...)
    # order; we define beta over that interleaved order.
    for lvl in range(NLEVELS):
        cf = coefs[lvl]                      # (n_l, K, 4)
        n_l = cf.shape[0]
        c0, c1, c2, c3 = cf[..., 0], cf[..., 1], cf[..., 2], cf[..., 3]
        bu = beta[0::2, :][:n_l, :]          # beta of a-child  (n_l, K)
        bv = beta[1::2, :][:n_l, :]          # beta of b-child
        with np.errstate(divide="ignore", invalid="ignore"):
            beta_normal = c1 * c2 / c3 - c0
        bad = (np.abs(c3) < C3_MIN) | (np.abs(beta_normal) > FOLD_TH)
        bad = ~np.isfinite(beta_normal) | bad
        flag = bad.any(axis=1)               # (n_l,) union over k
        flag_nk = np.broadcast_to(flag[:, None], c3.shape)

        s1 = c3.copy()
        b1 = c1 - c3 * bv
        with np.errstate(divide="ignore", invalid="ignore"):
            da_normal = c2 / c3 - bu
        da = np.where(flag_nk, -bu, da_normal)
        g = np.where(flag_nk, c2, 0.0)
        new_beta = np.where(flag_nk, c2 * bv - c0, beta_normal)

        flags.append(flag)
        params.append(dict(s1=s1, b1=b1, da=da, g=g))
        beta = new_beta                       # (n_l, K) -> next level children
    root_beta = beta[0]                       # (K,)
    return flags, params, root_beta


def _simulate(x, rh, rw, rc_a, rh_b, rw_b, rc_b, flags, params, root_beta,
              dtype=np.float32):
    """Host simulation of the exact device op sequence (for validation).
    rh/rw/rc_a are a-side (K,S); *_b the b-side."""
    xt = x.astype(dtype)
    # leaf planes (K, 2S, B, OUT, OUT) interleaved a/b
    leaves = np.empty((K, 2 * S, B, OUT, OUT), dtype=dtype)
    for k in range(K):
        for s in range(S):
            leaves[k, 2 * s] = xt[:, rc_a[k, s], rh[k, s]:rh[k, s] + OUT,
                                  rw[k, s]:rw[k, s] + OUT]
            leaves[k, 2 * s + 1] = xt[:, rc_b[k, s], rh_b[k, s]:rh_b[k, s] + OUT,
                                      rw_b[k, s]:rw_b[k, s] + OUT]
    cur = leaves                              # (K, 2n, B, OUT, OUT)
    for lvl in range(NLEVELS):
        pr = params[lvl]
        n_l = pr["s1"].shape[0]
        nxt = np.empty((K, n_l, B, OUT, OUT), dtype=dtype)
        for s in range(n_l):
            vu = cur[:, 2 * s]                # (K, B, OUT, OUT)
            vv = cur[:, 2 * s + 1]
            s1 = pr["s1"][s].astype(dtype)[:, None, None, None]
            b1 = pr["b1"][s].astype(dtype)[:, None, None, None]
            da = pr["da"][s].astype(dtype)[:, None, None, None]
            t1 = (vv * s1 + b1).astype(dtype)
            t2 = ((vu + da) * t1).astype(dtype)
            if flags[lvl][s]:
                gg = pr["g"][s].astype(dtype)[:, None, None, None]
                t2 = (vv * gg + t2).astype(dtype)
            nxt[:, s] = t2
        cur = nxt
    y = cur[:, 0] + (-root_beta.astype(dtype))[:, None, None, None]
    return np.transpose(y.astype(np.float32), (1, 0, 2, 3))   # (B, K, OUT, OUT)


# ------------------------------------------------------------ table builders


def _column_map(flags):
    """Deterministic dtab column layout; identical across cores because the
    flag structure is unioned over all k."""
    cols = {}
    nxt = 0
    for kl in range(KPC):
        for lvl in range(NLEVELS):
            n_l = S >> lvl
            for s in range(n_l):
                names = ("s1", "b1", "da") + (("g",) if flags[lvl][s] else ())
                for nm in names:
                    cols[(kl, lvl, s, nm)] = nxt
                    nxt += 1
        cols[(kl, "rootnb")] = nxt
        nxt += 1
    return cols, nxt


def _build_xr(x):
    """DRAM gather table [1920, 480]: row = ((rw*C + c)*2 + bhi)*H + h,
    col = bl*OUT + w;  value = x[bhi*8+bl, c, h, w+rw]."""
    xr = np.empty((RF, C, 2, H, 8, OUT), dtype=np.float32)
    for rw in range(RF):
        sl = x[:, :, :, rw:rw + OUT]                  # (B, C, H, OUT)
        xr[rw] = sl.reshape(2, 8, C, H, OUT).transpose(2, 0, 3, 1, 4)
    return np.ascontiguousarray(xr.reshape(XR_ROWS, 8 * OUT))


def _build_idx(core, rel_a, rel_b):
    """int32 [128, 128]: column j = kl*32 + side*16 + s holds per-partition
    row indices into xr; partition p = bhi*60 + oh."""
    rh_a, rw_a, rc_a = rel_a
    rh_b, rw_b, rc_b = rel_b
    idx = np.zeros((128, 128), dtype=np.int32)
    p = np.arange(PART)
    bhi, oh = p // OUT, p % OUT
    for kl in range(KPC):
        k = core * KPC + kl
        for side, (rh, rw, rc) in enumerate([(rh_a, rw_a, rc_a),
                                             (rh_b, rw_b, rc_b)]):
            for s in range(S):
                j = kl * 2 * S + side * S + s
                base = rw[k, s] * (C * 2 * H) + rc[k, s] * (2 * H)
                idx[:PART, j] = base + bhi * H + oh + rh[k, s]
    return idx


def _build_dtab(core, flags, params, root_beta, cols, ncols):
    dtab = np.zeros((128, ncols), dtype=np.float32)
    for kl in range(KPC):
        k = core * KPC + kl
        for lvl in range(NLEVELS):
            pr = params[lvl]
            n_l = S >> lvl
            for s in range(n_l):
                dtab[:, cols[(kl, lvl, s, "s1")]] = pr["s1"][s, k]
                dtab[:, cols[(kl, lvl, s, "b1")]] = pr["b1"][s, k]
                dtab[:, cols[(kl, lvl, s, "da")]] = pr["da"][s, k]
                if flags[lvl][s]:
                    dtab[:, cols[(kl, lvl, s, "g")]] = pr["g"][s, k]
        dtab[:, cols[(kl, "rootnb")]] = -root_beta[k]
    return dtab


# ------------------------------------------------------------- bass builder


def _kernel_body(tc, y_ap, xr_ap, idx_ap, dtab_ap, flags, cols, ncols):
    """Build the per-core program.  y: (B, KPC, OUT, OUT) DRAM out;
    xr: (1920, 480) DRAM; idx: (128, 128) i32 DRAM; dtab: (128, ncols) DRAM."""
    import concourse.mybir as mybir
    from concourse.bass import IndirectOffsetOnAxis
    from concourse.tile_rust import add_dep_helper  # noqa: F401

    nc = tc.nc
    f32 = mybir.dt.float32
    Ident = mybir.ActivationFunctionType.Identity
    ADD, MULT = mybir.AluOpType.add, mybir.AluOpType.mult

    with (
        tc.tile_pool(name="const", bufs=1) as const_pool,
        tc.tile_pool(name="leaf", bufs=56) as leaf_pool,
        tc.tile_pool(name="v", bufs=28) as v_pool,
        tc.tile_pool(name="t", bufs=6) as t_pool,
        tc.tile_pool(name="y", bufs=4) as y_pool,
    ):
        _emit(tc, nc, y_ap, xr_ap, idx_ap, dtab_ap, flags, cols, ncols,
              const_pool, leaf_pool, v_pool, t_pool, y_pool,
              f32, Ident, ADD, MULT, IndirectOffsetOnAxis)


def _emit(tc, nc, y_ap, xr_ap, idx_ap, dtab_ap, flags, cols, ncols,
          const_pool, leaf_pool, v_pool, t_pool, y_pool,
          f32, Ident, ADD, MULT, IndirectOffsetOnAxis):
    import concourse.mybir as mybir
    from concourse.tile_rust import add_dep_helper

    idx_sb = const_pool.tile([128, 128], mybir.dt.int32, tag="idx")
    dtab_sb = const_pool.tile([128, ncols], f32, tag="dtab")
    # Load the tables through the gpsimd SW queues so the HW queues are left
    # entirely to the output stores (whose first DMA then carries only its
    # single data wait).
    nc.gpsimd.dma_start(out=idx_sb[:], in_=idx_ap)
    nc.gpsimd.dma_start(out=dtab_sb[:], in_=dtab_ap)
    # Warm-up: make ScalarE and VectorE observe the dtab DMA semaphore once,
    # so the per-node ops below carry only their single producer wait
    # (Activation with AP scale+bias has room for just one sync-wait).
    nwarm = KPC * S  # plenty of distinct columns, one per warm-up op
    warm2 = const_pool.tile([128, nwarm + 1], f32, tag="warm2")
    nc.vector.tensor_copy(warm2[:, 0:1], dtab_sb[:, 0:1])
    warm_col = [1]  # next free column (0 used by the dtab warm above)

    def col(kl, lvl, s, nm):
        c = cols[(kl, lvl, s, nm)]
        return dtab_sb[:PART, c:c + 1]

    warm_g = const_pool.tile([128, KPC], f32, tag="warmg")
    warm_gy = const_pool.tile([128, KPC], f32, tag="warmgy")
    any_l0_flag = bool(np.any(flags[0]))
    tok_col = [None]                      # warm2 column of last DVE token
    for kl in range(KPC):
        # The leaf pool has 56 slots; kernel kl's 32 gathers reuse slots whose
        # previous readers are level-0 ops of kernels kl-1 (nodes <= 3) and
        # kl-2.  A tiny gpsimd op reading the previous kernel's DVE token
        # column (written right after level-0 node 3) makes the PL engine
        # observe a DVE tick covering all those readers, so each indirect DMA
        # below carries only its same-queue wait (these DMA encodings also
        # have a single wait slot).
        i_warm_g = None
        if tok_col[0] is not None:
            i_warm_g = nc.gpsimd.tensor_copy(
                warm_g[:, kl:kl + 1], warm2[:, tok_col[0]:tok_col[0] + 1])
        # ---- gather the 32 leaf planes of this kernel
        leaves = []                       # interleaved (a0, b0, a1, b1, ...)
        for s in range(S):
            pair = []
            for side in range(2):
                j = kl * 2 * S + side * S + s
                t = leaf_pool.tile([128, FREE], f32, tag="leaf")
                i_dma = nc.gpsimd.indirect_dma_start(
                    out=t[:PART],
                    out_offset=None,
                    in_=xr_ap,
                    in_offset=IndirectOffsetOnAxis(ap=idx_sb[:PART, j:j + 1],
                                                   axis=0),
                )
                if i_warm_g is not None:
                    add_dep_helper(i_dma.ins, i_warm_g.ins, sync=False)
                pair.append(t)
            leaves.extend(pair)
        # Warm-ups: the leaf DMAs round-robin over the 8 SWDGE queues (8
        # distinct semaphores).  Touch the last tile on every queue so the
        # vector engine observes each queue's final count; the per-node ops
        # below are then ordered after these (add_dep_helper) and carry at
        # most one sync-wait (their ISA encodings only have one wait slot).
        # All compute runs on DVE so every other dependency rides the single
        # DVE semaphore (merged into one wait).
        warm_dve = []
        for s in range(S - 4, S):
            c0 = warm_col[0]
            warm_col[0] += 2
            warm_dve.append(
                nc.vector.tensor_copy(warm2[:, c0:c0 + 1],
                                      leaves[2 * s][:, 0:1]))
            warm_dve.append(
                nc.vector.tensor_copy(warm2[:, c0 + 1:c0 + 2],
                                      leaves[2 * s + 1][:, 0:1]))
        cur = leaves
        # ---- tree levels
        for lvl in range(NLEVELS):
            n_l = S >> lvl
            nxt = []
            for s in range(n_l):
                vu, vv = cur[2 * s], cur[2 * s + 1]
                t1 = t_pool.tile([128, FREE], f32, tag="t1")
                i_t1 = nc.vector.tensor_scalar(
                    out=t1[:PART], in0=vv[:PART],
                    scalar1=col(kl, lvl, s, "s1"),
                    scalar2=col(kl, lvl, s, "b1"),
                    op0=MULT, op1=ADD)
                vt = v_pool.tile([128, FREE], f32, tag="v")
                i_stt = nc.vector.scalar_tensor_tensor(
                    out=vt[:PART], in0=vu[:PART],
                    scalar=col(kl, lvl, s, "da"), in1=t1[:PART],
                    op0=ADD, op1=MULT)
                if lvl == 0:
                    for wd in warm_dve:
                        add_dep_helper(i_t1.ins, wd.ins, sync=False)
                        add_dep_helper(i_stt.ins, wd.ins, sync=False)
                if flags[lvl][s]:
                    vt2 = v_pool.tile([128, FREE], f32, tag="v")
                    i_stt2 = nc.vector.scalar_tensor_tensor(
                        out=vt2[:PART], in0=vv[:PART],
                        scalar=col(kl, lvl, s, "g"), in1=vt[:PART],
                        op0=MULT, op1=ADD)
                    if lvl == 0:
                        for wd in warm_dve:
                            add_dep_helper(i_stt2.ins, wd.ins, sync=False)
                    vt = vt2
                nxt.append(vt)
                if lvl == 0 and s == 3:
                    # DVE token: a later gpsimd read of this unique column
                    # observes a DVE tick past all level-0 nodes <= 3.
                    c0 = warm_col[0]
                    warm_col[0] += 1
                    nc.vector.tensor_copy(warm2[:, c0:c0 + 1], vt[:, 0:1])
                    tok_col[0] = c0
            cur = nxt
        # ---- root: subtract beta, store
        yt = y_pool.tile([128, FREE], f32, tag="y")
        rc = cols[(kl, "rootnb")]
        nc.vector.tensor_scalar(
            out=yt[:PART], in0=cur[0][:PART],
            scalar1=dtab_sb[:PART, rc:rc + 1], scalar2=None, op0=ADD)
        # Root token + gpsimd observation, so the output stores (also on the
        # SW queues -- keeping the drain's wait list small) carry only their
        # in-queue wait.
        c0 = warm_col[0]
        warm_col[0] += 1
        nc.vector.tensor_copy(warm2[:, c0:c0 + 1], yt[:, 0:1])
        i_warm_y = nc.gpsimd.tensor_copy(warm_gy[:, kl:kl + 1],
                                         warm2[:, c0:c0 + 1])
        for bh in range(2):
            dst = y_ap[bh * 8:(bh + 1) * 8, kl].rearrange("bl o w -> o bl w")
            src = yt[bh * OUT:(bh + 1) * OUT].rearrange("o (bl w) -> o bl w",
                                                        bl=8)
            i_st = nc.gpsimd.dma_start(out=dst, in_=src)
            add_dep_helper(i_st.ins, i_warm_y.ins, sync=False)


def _prepare(x, ah, aw, ac, bh, bw, bc, ws):
    rel_a = _derive_rel(ah, aw, ac)
    rel_b = _derive_rel(bh, bw, bc)
    coefs = _coef_levels(ws)
    flags, params, root_beta = _fold_tree(coefs)
    cols, ncols = _column_map(flags)
    xr = _build_xr(np.asarray(x, dtype=np.float32))
    idxs = [_build_idx(c, rel_a, rel_b) for c in range(NCORES)]
    dtabs = [_build_dtab(c, flags, params, root_beta, cols, ncols)
             for c in range(NCORES)]
    return flags, cols, ncols, xr, idxs, dtabs


# ----------------------------------------------------------------- runner

_CACHE = {}
_DRAIN_PATCHED = False


def _apply_drain_patch():
    """The walrus build in this container allows only ONE sync-wait per
    instruction, but TileContext's tail drain carries one wait per
    engine/DMA-queue.  Split the extras onto standalone single-wait SP nops
    (the same shape the barrier butterfly uses)."""
    global _DRAIN_PATCHED
    if _DRAIN_PATCHED:
        return
    import concourse.mybir as mybir
    import concourse.tile as tile_mod
    from concourse.vector_clock import ScopedClock

    def _patched(self, tick_clock, wait_clock):
        drain_inst = self.nc.sync.drain()
        wait_clock.add_sem_waits(
            drain_inst.ins, ScopedClock({None: tick_clock.global_clock})
        )
        si = drain_inst.ins.sync_info
        if si is not None and si.on_wait and len(si.on_wait) > 1:
            waits = list(si.on_wait)
            si.on_wait = waits[:1]
            for w in waits[1:]:
                nop = self.nc.sync.nop(nofuse=True)
                nsi = nop.ins.sync_info
                if nsi is None:
                    nop.ins.sync_info = mybir.SyncInfo(on_wait=[w],
                                                       on_update=[])
                else:
                    nsi.on_wait = [w]

        self.nc.all_engine_barrier()
        assert self.sems is not None
        popped = self.nc._tile_sem_poison_stack.pop()
        assert popped is self._sem_poison
        self.nc.clear_and_free_semaphores(list(self.sems.allocated().values()))
        self.nc.all_engine_barrier()

    tile_mod.TileContext._drain_and_barrier = _patched
    _DRAIN_PATCHED = True


def _get_compiled(flags, cols, ncols):
    key = ("k", tuple(tuple(bool(b) for b in f) for f in flags), ncols)
    if key in _CACHE:
        return _CACHE[key]
    import concourse.bass as bass
    import concourse.mybir as mybir
    from concourse.tile import TileContext

    _apply_drain_patch()
    nc = bass.Bass("TRN2")
    f32 = mybir.dt.float32
    xr_t = nc.dram_tensor("xr", [XR_ROWS, 8 * OUT], f32, kind="ExternalInput")
    idx_t = nc.dram_tensor("idx", [128, 128], mybir.dt.int32,
                           kind="ExternalInput")
    dtab_t = nc.dram_tensor("dtab", [128, ncols], f32, kind="ExternalInput")
    y_t = nc.dram_tensor("y", [B, KPC, OUT, OUT], f32, kind="ExternalOutput")
    with TileContext(nc) as tc:
        _kernel_body(tc, y_t.ap(), xr_t.ap(), idx_t.ap(), dtab_t.ap(),
                     flags, cols, ncols)
    _CACHE[key] = nc
    return nc


TRACE = False          # set True (e.g. from test.py) to capture an NTFF trace
LAST_RESULTS = None    # BassKernelResults of the most recent kernel() call


def kernel(x, ah, aw, ac, bh, bw, bc, w0, w1, w2, w3, w4):
    global LAST_RESULTS
    from concourse import bass_utils

    flags, cols, ncols, xr, idxs, dtabs = _prepare(
        x, ah, aw, ac, bh, bw, bc, [w0, w1, w2, w3, w4])
    nc = _get_compiled(flags, cols, ncols)
    in_maps = [{"xr": xr, "idx": idxs[c], "dtab": dtabs[c]}
               for c in range(NCORES)]
    res = bass_utils.run_bass_kernel_spmd(nc, in_maps,
                                          core_ids=list(range(NCORES)),
                                          trace=TRACE)
    LAST_RESULTS = res
    y = np.empty((B, K, OUT, OUT), dtype=np.float32)
    for c in range(NCORES):
        y[:, c * KPC:(c + 1) * KPC] = res.results[c]["y"]
    return y
